# revision 2
# baseline (speedup 1.0000x reference)
"""Bilinear interpolation (affine grid sample) TRN2 Bass kernel, v2.

Differences vs v1 (kernel.py):
  - Gather source GY is bf16 in "y-group interleaved" layout:
      GY[h, x, yi, c] = X[c, 2h + yi, x], h in [0,128), yi in [0,4)
    (group h covers image rows [2h, 2h+4); consecutive groups overlap by
    2 rows, so each row is stored twice = 16.8 MB per batch).
  - ONE gather window per point (vs two): window = 512 bf16 el = 1 KB at
    record u = (y0>>1)*256 + x0, covering cols {x0, x0+1} x rows
    {2h..2h+3}.  All y0 parities handled by the yi-blended weights.
  - Combine runs in bf16 with 5 wide DVE ops per chunk (2 broadcast muls
    + 3 dense adds) instead of 11 narrow ones.
  - Phase A writes GY with 512B dest runs; loads cast f32->bf16 in DMA.
"""

import sys

sys.path.insert(0, "/opt/trn_rl_repo")

import numpy as np
import ml_dtypes  # noqa: F401

import concourse.bass as bass
import concourse.bacc as bacc
import concourse.mybir as mybir
from concourse import tile
from concourse.bass import AP
from concourse.masks import make_identity

F32 = mybir.dt.float32
BF16 = mybir.dt.bfloat16
I16 = mybir.dt.int16
I32 = mybir.dt.int32

B, C, H, W = 16, 64, 256, 256
HW = H * W
NCORES = 8
BPC = B // NCORES  # 2

NPTS = HW
NGRP = NPTS // 128  # 512
JCHUNK = 16
NCHUNK = NGRP // JCHUNK
CHUNK_PTS = JCHUNK * 128

NREC = 32768  # records of 256 bf16 el (512 B); u = (y0>>1)*256 + x0
RECEL = 256  # elements per record step
WINEL = 512  # gather window: 2 records = 8 px = 1 KB
GYSZ = 128 * 65536 + WINEL  # + pad so the u=32767 AP row stays in-bounds

SCALE = 2.0 * (W / 2) / (W - 1)  # 256/255


def _host_consts():
    q = np.arange(128)
    s = np.arange(NPTS // 16)
    j = np.arange(NGRP)
    p = np.arange(128)
    # wrapped idx layout: gather position i = 16*s + (q%16); point m = i
    ixw = 16.0 * (s % 16)[None, :] + (q % 16)[:, None]
    iyw = np.broadcast_to((s // 16)[None, :], (128, len(s))).astype(np.float64)
    # weight layout: point m = j*128 + p
    pjx = 128.0 * (j % 2)[None, :] + p[:, None]
    pjy = np.broadcast_to((j // 2)[None, :], (128, NGRP)).astype(np.float64)
    return {
        "IXW": np.ascontiguousarray(ixw, "bfloat16"),
        "IYW": np.ascontiguousarray(iyw, "bfloat16"),
        "PJX": np.ascontiguousarray(pjx, "bfloat16"),
        "PJY": np.ascontiguousarray(pjy, "bfloat16"),
    }


def build_nc(n_batches=BPC, n_chunks=NCHUNK, debug=False):
    nc = bacc.Bacc("TRN2", target_bir_lowering=False, debug=debug)

    x_in = nc.declare_dram_parameter("X", [n_batches, C, HW], F32, isOutput=False)
    th_in = nc.declare_dram_parameter("THETA", [n_batches, 128, 6], F32, isOutput=False)
    ixw_in = nc.declare_dram_parameter("IXW", [128, NPTS // 16], BF16, isOutput=False)
    iyw_in = nc.declare_dram_parameter("IYW", [128, NPTS // 16], BF16, isOutput=False)
    pjx_in = nc.declare_dram_parameter("PJX", [128, NGRP], BF16, isOutput=False)
    pjy_in = nc.declare_dram_parameter("PJY", [128, NGRP], BF16, isOutput=False)
    out_ext = nc.declare_dram_parameter("OUT", [n_batches, C, HW], F32, isOutput=True)

    gys = [nc.dram_tensor(f"gy{b}", [GYSZ], BF16) for b in range(n_batches)]

    _regcache = {}

    with tile.TileContext(nc) as tc:
        import contextlib

        with contextlib.ExitStack() as ctx:
            cpool = ctx.enter_context(tc.tile_pool(name="consts", bufs=1))
            xpool = ctx.enter_context(tc.tile_pool(name="xload", bufs=2))
            apsum = ctx.enter_context(tc.tile_pool(name="tpsum", bufs=2, space="PSUM"))
            hpool = ctx.enter_context(tc.tile_pool(name="hsbuf", bufs=2))
            wpool = ctx.enter_context(tc.tile_pool(name="weights", bufs=1))
            spool = ctx.enter_context(tc.tile_pool(name="scratch", bufs=1))
            gpool = ctx.enter_context(tc.tile_pool(name="gather", bufs=2))
            mpool = ctx.enter_context(tc.tile_pool(name="mulbuf", bufs=1))
            opool = ctx.enter_context(tc.tile_pool(name="outbuf", bufs=2))
            opsum = ctx.enter_context(tc.tile_pool(name="opsum", bufs=2, space="PSUM"))

            identb = cpool.tile([128, 128], BF16)
            make_identity(nc, identb[:])
            ixw = cpool.tile([128, NPTS // 16], BF16)
            nc.sync.dma_start(out=ixw[:], in_=ixw_in.ap())
            iyw = cpool.tile([128, NPTS // 16], BF16)
            nc.sync.dma_start(out=iyw[:], in_=iyw_in.ap())
            pjx = cpool.tile([128, NGRP], BF16)
            nc.sync.dma_start(out=pjx[:], in_=pjx_in.ap())
            pjy = cpool.tile([128, NGRP], BF16)
            nc.sync.dma_start(out=pjy[:], in_=pjy_in.ap())

            ztail = cpool.tile([1, WINEL], BF16)
            nc.vector.memset(ztail[:], 0.0)
            for _gy in gys:
                nc.sync.dma_start(
                    out=AP(_gy.ap().tensor, GYSZ - WINEL, [[1, WINEL]]),
                    in_=ztail[:],
                )

            V = nc.vector
            S = nc.scalar

            def tsc(out, in0, s1, op0, s2=None, op1=None):
                if s2 is None:
                    return V.tensor_scalar(out, in0, s1, None, op0)
                return V.tensor_scalar(out, in0, s1, s2, op0, op1)

            A = mybir.AluOpType

            for b in range(n_batches):
                xb = x_in.ap()[b]  # [64, HW] f32
                ob = out_ext.ap()[b]
                gy = gys[b].ap()  # [GYSZ] bf16

                # ---------- Phase A: build GY ----------
                # xinb tiles: 8 x [64, 8192] bf16 (cast in DMA), 32 rows each
                xtiles = []
                for k in range(8):
                    xt = xpool.tile([C, 8192], BF16, tag="xinb")
                    nc.gpsimd.dma_start(
                        out=xt[:], in_=xb[:, k * 8192 : (k + 1) * 8192]
                    )
                    xtiles.append(xt)

                # hs tiles: each covers 4 groups [128, 4*512] bf16
                # group g content: free = q*512 + xh*256 + yi*64 + c,
                # partition = x & 127 (x = xh*128 + p)
                hs_tiles = [None] * 32

                def hs_tile(k):
                    if hs_tiles[k] is None:
                        hs_tiles[k] = hpool.tile([128, 2048], BF16, tag="hs", name=f"hs{k}")
                    return hs_tiles[k]

                def flush_hs(k):
                    ht = hs_tiles[k]
                    assert ht is not None
                    src = ht[:].rearrange(
                        "p (q xh e) -> p q xh e", q=4, xh=2
                    )  # e = yi*64+c = 256
                    dst = AP(
                        gy.tensor,
                        4 * k * 65536,
                        [[256, 128], [65536, 4], [32768, 2], [1, 256]],
                    )
                    nc.sync.dma_start(out=dst, in_=src)
                    hs_tiles[k] = None

                for P in range(128):  # row pair P: rows (2P, 2P+1)
                    ps = apsum.tile([128, 256], BF16, tag="psA")
                    for i in range(4):
                        r = 2 * P + (i >> 1)
                        xh = i & 1
                        xt = xtiles[r // 32]
                        sl = (r % 32) * 256 + xh * 128
                        nc.tensor.transpose(
                            ps[:, i * 64 : (i + 1) * 64],
                            xt[:, sl : sl + 128],
                            identb[:C, :C],
                        )
                    # copy1: group P, yi in {0,1}
                    ht = hs_tile(P // 4)
                    q = P % 4
                    dst1 = AP(
                        ht[:].tensor,
                        ht[:].offset + q * 512,
                        [ht[:].ap[0], [64, 2], [256, 2], [1, 64]],
                    )
                    src1 = ps[:].rearrange("p (yi xh c) -> p yi xh c", yi=2, xh=2)
                    S.copy(dst1, src1)
                    # copy2: group P-1, yi in {2,3}
                    if P >= 1:
                        ht2 = hs_tile((P - 1) // 4)
                        q2 = (P - 1) % 4
                        dst2 = AP(
                            ht2[:].tensor,
                            ht2[:].offset + q2 * 512 + 128,
                            [ht2[:].ap[0], [64, 2], [256, 2], [1, 64]],
                        )
                        S.copy(dst2, src1)
                        if q2 == 3:
                            flush_hs((P - 1) // 4)
                # group 127 yi in {2,3}: rows 256/257 don't exist -> zero
                ht = hs_tile(31)
                dstz = AP(
                    ht[:].tensor,
                    ht[:].offset + 3 * 512 + 128,
                    [ht[:].ap[0], [256, 2], [1, 128]],
                )
                V.memset(dstz, 0.0)
                flush_hs(31)

                # ---------- Phase B: theta-derived scalars ----------
                thsb = spool.tile([128, 6], F32, tag="thsb")
                nc.sync.dma_start(out=thsb[:], in_=th_in.ap()[b])
                thb = thsb
                sc = spool.tile([128, 8], F32, tag="thsc")
                tsc(sc[:, 0:1], thb[:, 0:1], SCALE, A.mult)
                tsc(sc[:, 1:2], thb[:, 1:2], SCALE, A.mult)
                V.tensor_tensor(sc[:, 2:3], thb[:, 2:3], thb[:, 0:1], A.subtract)
                V.tensor_tensor(sc[:, 2:3], sc[:, 2:3], thb[:, 1:2], A.subtract)
                tsc(sc[:, 2:3], sc[:, 2:3], 1.0, A.add, float(W // 2), A.mult)
                tsc(sc[:, 3:4], thb[:, 3:4], SCALE, A.mult)
                tsc(sc[:, 4:5], thb[:, 4:5], SCALE, A.mult)
                V.tensor_tensor(sc[:, 5:6], thb[:, 5:6], thb[:, 3:4], A.subtract)
                V.tensor_tensor(sc[:, 5:6], sc[:, 5:6], thb[:, 4:5], A.subtract)
                tsc(sc[:, 5:6], sc[:, 5:6], 1.0, A.add, float(H // 2), A.mult)
                ax, bx, cx = sc[:, 0:1], sc[:, 1:2], sc[:, 2:3]
                ay, by, cy = sc[:, 3:4], sc[:, 4:5], sc[:, 5:6]

                # ---------- Phase C: gather indices [128, 4096] i16 ----------
                SW = NPTS // 16
                NSPL = 4
                SH = SW // NSPL
                idx16 = wpool.tile([128, SW], I16)
                for hh in range(NSPL):
                    hsl = slice(hh * SH, (hh + 1) * SH)
                    t0 = spool.tile([128, SH], F32, tag="wk0")
                    t1 = spool.tile([128, SH], F32, tag="wk1")
                    i0 = spool.tile([128, SH], I32, tag="wki0")
                    i1 = spool.tile([128, SH], I32, tag="wki1")
                    # x0c = floor(clamp(x, 0, 254))
                    tsc(t0[:], ixw[:, hsl], ax, A.mult)
                    V.scalar_tensor_tensor(t0[:], iyw[:, hsl], bx, t0[:], A.mult, A.add)
                    tsc(t0[:], t0[:], cx, A.add, 0.0, A.max)
                    tsc(t0[:], t0[:], float(W - 2), A.min, 0.5, A.subtract)
                    V.tensor_copy(i0[:], t0[:])  # RNE -> floor
                    # y0c = floor(clamp(y, 0, 254)); idx = (y0c>>1)<<8 | x0c
                    tsc(t1[:], ixw[:, hsl], ay, A.mult)
                    V.scalar_tensor_tensor(t1[:], iyw[:, hsl], by, t1[:], A.mult, A.add)
                    tsc(t1[:], t1[:], cy, A.add, 0.0, A.max)
                    tsc(t1[:], t1[:], float(H - 2), A.min, 0.5, A.subtract)
                    V.tensor_copy(i1[:], t1[:])
                    tsc(i1[:], i1[:], 1, A.arith_shift_right)
                    tsc(i1[:], i1[:], 8, A.logical_shift_left)
                    V.tensor_tensor(i1[:], i1[:], i0[:], A.add)
                    V.tensor_copy(idx16[:, hsl], i1[:])

                # ---------- Phase D: 8-slot weights [128, NGRP*8] ----------
                W8 = wpool.tile([128, NGRP * 8], F32, tag="W8")
                W8b = wpool.tile([128, NGRP * 8], BF16, tag="W8b")
                W8v = W8[:].rearrange("p (j s) -> p j s", s=8)

                xv = spool.tile([128, NGRP], F32, tag="xv")
                yv = spool.tile([128, NGRP], F32, tag="yv")
                u0 = spool.tile([128, NGRP], F32, tag="u0")
                u1 = spool.tile([128, NGRP], F32, tag="u1")
                u2 = spool.tile([128, NGRP], F32, tag="u2")
                u3 = spool.tile([128, NGRP], F32, tag="u3")
                iw0 = spool.tile([128, NGRP], I32, tag="iw0")
                tsc(xv[:], pjx[:], ax, A.mult)
                V.scalar_tensor_tensor(xv[:], pjy[:], bx, xv[:], A.mult, A.add)
                tsc(xv[:], xv[:], cx, A.add)
                tsc(yv[:], pjx[:], ay, A.mult)
                V.scalar_tensor_tensor(yv[:], pjy[:], by, yv[:], A.mult, A.add)
                tsc(yv[:], yv[:], cy, A.add)
                # valid mask -> u0
                tsc(u0[:], xv[:], 0.0, A.is_ge)
                tsc(u1[:], xv[:], float(W - 1), A.is_lt)
                V.tensor_tensor(u0[:], u0[:], u1[:], A.mult)
                tsc(u1[:], yv[:], 0.0, A.is_ge)
                V.tensor_tensor(u0[:], u0[:], u1[:], A.mult)
                tsc(u1[:], yv[:], float(H - 1), A.is_lt)
                V.tensor_tensor(u0[:], u0[:], u1[:], A.mult)
                # x side: u1 = x+ ; u2 = fx
                tsc(u1[:], xv[:], 0.0, A.max)
                tsc(u2[:], u1[:], 0.5, A.subtract)
                V.tensor_copy(iw0[:], u2[:])
                V.tensor_copy(u2[:], iw0[:])
                V.tensor_tensor(u2[:], u1[:], u2[:], A.subtract)  # fx
                # wxv0 = (1-fx)*valid -> xv ; wxv1 = fx*valid -> u2
                tsc(xv[:], u2[:], -1.0, A.mult, 1.0, A.add)
                V.tensor_tensor(xv[:], xv[:], u0[:], A.mult)
                V.tensor_tensor(u2[:], u2[:], u0[:], A.mult)
                # y side: u1 = y+ ; yv = fy ; u3 = pary
                tsc(u1[:], yv[:], 0.0, A.max)
                tsc(yv[:], u1[:], 0.5, A.subtract)
                V.tensor_copy(iw0[:], yv[:])
                V.tensor_copy(yv[:], iw0[:])
                V.tensor_tensor(yv[:], u1[:], yv[:], A.subtract)  # fy
                tsc(iw0[:], iw0[:], 1, A.bitwise_and)
                V.tensor_copy(u3[:], iw0[:])  # pary
                # u1 = 1-pary (parc), u0 free after folding into wx
                parc = u1
                tsc(parc[:], u3[:], -1.0, A.mult, 1.0, A.add)
                wy0 = u0  # reuse: wy0 = 1-fy (valid already folded into wx)
                tsc(wy0[:], yv[:], -1.0, A.mult, 1.0, A.add)
                # wys0 = wy0*parc; wys1 = wy0*pary + fy*parc; wys2 = fy*pary
                wys0 = spool.tile([128, NGRP], F32, tag="wys0")
                wys1 = spool.tile([128, NGRP], F32, tag="wys1")
                wys2 = spool.tile([128, NGRP], F32, tag="wys2")
                V.tensor_tensor(wys0[:], wy0[:], parc[:], A.mult)
                V.tensor_tensor(wys1[:], wy0[:], u3[:], A.mult)
                V.tensor_tensor(parc[:], yv[:], parc[:], A.mult)
                V.tensor_tensor(wys1[:], wys1[:], parc[:], A.add)
                V.tensor_tensor(wys2[:], yv[:], u3[:], A.mult)
                # W8 slots: dx*4 + yi; slots 3,7 = 0
                V.memset(W8[:], 0.0)
                V.tensor_tensor(W8v[:, :, 0], xv[:], wys0[:], A.mult)
                V.tensor_tensor(W8v[:, :, 1], xv[:], wys1[:], A.mult)
                V.tensor_tensor(W8v[:, :, 2], xv[:], wys2[:], A.mult)
                V.tensor_tensor(W8v[:, :, 4], u2[:], wys0[:], A.mult)
                V.tensor_tensor(W8v[:, :, 5], u2[:], wys1[:], A.mult)
                V.tensor_tensor(W8v[:, :, 6], u2[:], wys2[:], A.mult)
                V.tensor_copy(W8b[:], W8[:])
                W8bv = W8b[:].rearrange("p (j s) -> p j s", s=8)

                # ---------- Phase E: gather + combine + transpose + out ----
                in_gy = AP(gy.tensor, 0, [[RECEL, NREC], [1, WINEL]])
                if "nreg" not in _regcache:
                    _regcache["nreg"] = nc.gpsimd.to_reg(CHUNK_PTS)
                nreg = _regcache["nreg"]
                for ci in range(n_chunks):
                    g = gpool.tile([128, JCHUNK, WINEL], BF16, tag="g")
                    idxs = idx16[
                        :, ci * (CHUNK_PTS // 16) : (ci + 1) * (CHUNK_PTS // 16)
                    ]
                    nc.gpsimd.dma_gather(
                        g[:], in_gy, idxs, CHUNK_PTS, nreg, WINEL,
                        elem_step=RECEL, queue_num=0, single_packet=False,
                    )
                    gv = g[:].rearrange("p j (s c) -> p j s c", c=64)
                    cj = slice(ci * JCHUNK, (ci + 1) * JCHUNK)
                    w4a = (
                        W8bv[:, cj, 0:4].unsqueeze(3)
                        .to_broadcast([128, JCHUNK, 4, 64])
                    )
                    w4b = (
                        W8bv[:, cj, 4:8].unsqueeze(3)
                        .to_broadcast([128, JCHUNK, 4, 64])
                    )
                    m1 = mpool.tile([128, JCHUNK, 4, 64], BF16, tag="m1")
                    m2 = mpool.tile([128, JCHUNK, 4, 64], BF16, tag="m2")
                    V.tensor_tensor(m1[:], gv[:, :, 0:4, :], w4a, A.mult)
                    V.tensor_tensor(m2[:], gv[:, :, 4:8, :], w4b, A.mult)
                    V.tensor_tensor(m1[:], m1[:], m2[:], A.add)
                    m1v = m1[:].rearrange("p j s c -> p j (s c)")
                    s2 = mpool.tile([128, JCHUNK, 128], BF16, tag="s2")
                    V.tensor_tensor(s2[:], m1v[:, :, 0:128], m1v[:, :, 128:256], A.add)
                    comb = opool.tile([128, JCHUNK, 64], BF16, tag="comb")
                    V.tensor_tensor(comb[:], s2[:, :, 0:64], s2[:, :, 64:128], A.add)

                    # transpose [pt, c] -> [c, pt]: J/2 transposes of [128,128]
                    ps = opsum.tile([128, (JCHUNK // 2) * 128], BF16, tag="psO")
                    combv = comb[:].rearrange("p j c -> p (j c)")
                    for t in range(JCHUNK // 2):
                        nc.tensor.transpose(
                            ps[:, t * 128 : (t + 1) * 128],
                            combv[:, t * 128 : (t + 1) * 128],
                            identb[:],
                        )
                    # psum [(j&1)*64+c, (j>>1)*128+pt] -> outsb [c, j*128+pt]
                    outsb = opool.tile([C, JCHUNK * 128], F32, tag="outsb")
                    psv = ps[:].rearrange("p (t q) -> p t q", q=128)
                    osv = outsb[:].rearrange("c (j q) -> c j q", q=128)
                    S.copy(osv[:, 0::2, :], psv[0:64, :, :])
                    S.copy(osv[:, 1::2, :], psv[64:128, :, :])
                    nc.sync.dma_start(
                        out=ob[:, ci * CHUNK_PTS : (ci + 1) * CHUNK_PTS],
                        in_=outsb[:],
                    )

    nc.compile()
    return nc


_CONSTS = _host_consts()


def _make_in_maps(X, theta, n_batches=BPC):
    Xr = np.ascontiguousarray(X.reshape(B, C, HW), np.float32)
    th = np.ascontiguousarray(theta, np.float32)
    in_maps = []
    for core in range(NCORES):
        b0 = core * n_batches
        th_rep = np.repeat(th[b0 : b0 + n_batches, None, :], 128, axis=1)
        in_maps.append(
            {
                "X": Xr[b0 : b0 + n_batches],
                "THETA": np.ascontiguousarray(th_rep, np.float32),
                **_CONSTS,
            }
        )
    return in_maps


_NC_CACHE = {}


def kernel(X, affine_transformation):
    from concourse.bass_utils import run_bass_kernel_spmd

    X = np.asarray(X, np.float32)
    theta = np.asarray(affine_transformation, np.float32)
    if "nc" not in _NC_CACHE:
        _NC_CACHE["nc"] = build_nc()
    nc = _NC_CACHE["nc"]
    in_maps = _make_in_maps(X, theta)
    res = run_bass_kernel_spmd(nc, in_maps, list(range(NCORES)))
    outs = [r["OUT"].reshape(BPC, C, H, W) for r in res.results]
    return np.concatenate(outs, axis=0)


if __name__ == "__main__":
    mode = sys.argv[1] if len(sys.argv) > 1 else "sim"
    if mode == "build":
        nc = build_nc()
        print("build ok")
    elif mode == "sim":
        n_chunks = int(sys.argv[2]) if len(sys.argv) > 2 else 2
        import concourse.bass_interp as bass_interp

        _orig_copy = bass_interp.InstructionExecutor.visit_InstTensorCopy

        def _copy_rne(self, instruction, *, reg_snapshot=None):
            from concourse.bass_interp import Direction, InterpAPClass

            inp, outp = instruction.ins[0], instruction.outs[0]
            if isinstance(inp, InterpAPClass) and isinstance(outp, InterpAPClass):
                iv = self.view_ap(
                    inp, Direction.READ, instruction, reg_snapshot=reg_snapshot
                )
                ov = self.view_ap(
                    outp, Direction.WRITE, instruction, reg_snapshot=reg_snapshot
                )
                if np.issubdtype(iv.dtype, np.floating) and np.issubdtype(
                    ov.dtype, np.integer
                ):
                    ov[:] = np.round(iv.reshape(ov.shape))
                    return
            return _orig_copy(self, instruction, reg_snapshot=reg_snapshot)

        bass_interp.InstructionExecutor.visit_InstTensorCopy = _copy_rne

        rng = np.random.default_rng(0)
        Xt = rng.standard_normal((1, C, HW), dtype=np.float32)
        th = rng.standard_normal((1, 6), dtype=np.float32) * 0.7
        nc = build_nc(n_batches=1, n_chunks=n_chunks, debug=False)
        th_rep = np.repeat(th[:, None, :], 128, axis=1)
        sim = bass_interp.CoreSim(nc)
        sim.tensor("X")[:] = Xt
        sim.tensor("THETA")[:] = np.ascontiguousarray(th_rep, np.float32)
        for k, v in _CONSTS.items():
            sim.tensor(k)[:] = v
        sim.simulate()
        got = np.array(sim.tensor("OUT"))

        def ref(Xf, thf):
            xl = np.linspace(-1, 1, W, dtype=np.float32)
            yl = np.linspace(-1, 1, H, dtype=np.float32)
            xc, yc = np.meshgrid(xl, yl, indexing="ij")
            grid = np.stack([xc.ravel(), yc.ravel(), np.ones(W * H, np.float32)], 0)
            thr = thf.reshape(-1, 2, 3)
            sampled = np.einsum("bij,jn->bin", thr, grid)
            x = (sampled[:, 0, :] + 1) * (W * 0.5)
            y = (sampled[:, 1, :] + 1) * (H * 0.5)
            x0 = np.clip(np.floor(x).astype(np.int64), 0, W - 1)
            x1 = np.clip(np.floor(x).astype(np.int64) + 1, 0, W - 1)
            y0 = np.clip(np.floor(y).astype(np.int64), 0, H - 1)
            y1 = np.clip(np.floor(y).astype(np.int64) + 1, 0, H - 1)
            flat = Xf.reshape(-1, C, H * W).transpose(0, 2, 1)
            bidx = np.arange(flat.shape[0])[:, None]
            pa = flat[bidx, y0 * W + x0]
            pb = flat[bidx, y1 * W + x0]
            pc = flat[bidx, y0 * W + x1]
            pd = flat[bidx, y1 * W + x1]
            x0f, x1f, y0f, y1f = (a.astype(np.float32) for a in (x0, x1, y0, y1))
            wa = ((x1f - x) * (y1f - y))[..., None]
            wb = ((x1f - x) * (y - y0f))[..., None]
            wc = ((x - x0f) * (y1f - y))[..., None]
            wd = ((x - x0f) * (y - y0f))[..., None]
            out = wa * pa + wb * pb + wc * pc + wd * pd
            return out.reshape(-1, W, H, C).transpose(0, 3, 2, 1)

        exp_full = ref(Xt, th).reshape(1, C, HW)
        npts = n_chunks * CHUNK_PTS
        got_s = got[0][:, :npts]
        exp_s = exp_full[0][:, :npts]
        err = np.abs(got_s - exp_s)
        denom = np.abs(exp_s).max() + 1e-8
        print("max abs err:", err.max(), " max |exp|:", np.abs(exp_s).max())
        print(
            "rel l2:",
            np.linalg.norm(got_s - exp_s) / (np.linalg.norm(exp_s) + 1e-8),
        )
        bad = np.argwhere(err > 3e-2 * denom)
        print("n bad:", len(bad), "of", got_s.size)
        if len(bad):
            print("first bad:", bad[:5])
